# revision 15
# baseline (speedup 1.0000x reference)
"""XNOR-Net++ 3x3 conv (sign(x) (*) sign(w) * alpha*beta*gamma) on 8 TRN2 NeuronCores.

Sharding: data-parallel over batch (32 -> 4 per core), weights/scales replicated.

v7 design (trace-driven; baseline 177us -> 143 -> 128 -> 123):
- host packs sign(weight) into the transposed fp8 DoubleRow layout (split per
  ob so only the first half gates the start) and folds alpha*beta*gamma into
  per-ob f32 scale maps (weight folding)
- zero-padded fp8 sign images with 57-stride rows (adjacent rows share a pad
  column): conv taps are flat offsets ky*57+kx, each matmul N=455 covers 8
  output rows; pads zeroed once at startup, signs write interiors only
- each image is split into seven row-band tiles (one per row-tile of 8
  output rows) with matching split x tiles; the tile framework tracks deps
  per tile, so the first matmul gates on 0.52MB of DMA + two 0.7us signs and
  afterwards bands arrive (1.3us DMA + 1.4us sign) faster than the 1.73us
  the PE spends per tile -- no pipeline gaps
- x for image b+1 is issued before image b's compute so the output triggers
  (which wait on epilogues) never delay it in the sync queue's FIFO
- outputs ship from a bf16 staging tile in 2 chunks per (image, ob) on the
  sync queue (the gpsimd queue's end-of-kernel DRAIN costs ~100ns/descriptor)
- PE pre-warm: junk fp8 matmuls bridge the head so the HAM clock gate is at
  2.4GHz when the real stream starts; a dummy sign preloads the ACT table
- PE then runs only the 504 conv matmuls back-to-back at ~192ns each
"""

from contextlib import ExitStack

import numpy as np

import concourse.bacc as bacc
import concourse.mybir as mybir
import concourse.tile as tile
from concourse.bass_utils import run_bass_kernel_spmd

N_CORES = 8
B, C, H, KS = 32, 256, 56, 3
P = 128
CB = C // P  # input-channel blocks (2)
OB = C // P  # output-channel blocks (2)
W1 = H + 1   # row stride (57): right pad of row r == left pad of row r+1
R = 8        # output rows per matmul tile
T = H // R   # row tiles per image (7)
NT = (R - 1) * W1 + H  # moving free dim per matmul (455)
HW = H * H   # pixels per image (3136)
HP = H + 2   # padded rows (58)

# image row-bands, one per row-tile: band t covers padded rows [8t, 8t+10)
BANDS = [(8 * t, 8 * t + 10) for t in range(T)]
BAND_OF_T = list(range(T))


def _pad16(n):
    return (n + 15) // 16 * 16


F32 = mybir.dt.float32
BF16 = mybir.dt.bfloat16
FP8 = mybir.dt.float8e4
DR = mybir.MatmulPerfMode.DoubleRow

NP_FP8 = mybir.dt.np(FP8)

N_WARM = 10  # pre-warm matmuls (N=512 fp8, ~4us at cold clock)


def build_conv(tc, out_ap, x_ap, w_ap, s_ap, BL):
    nc = tc.nc
    with ExitStack() as ctx:
        const_pool = ctx.enter_context(tc.tile_pool(name="const", bufs=1))
        xpool = ctx.enter_context(tc.tile_pool(name="x", bufs=2))
        imgpool = ctx.enter_context(tc.tile_pool(name="img", bufs=2))
        psumpool = ctx.enter_context(tc.tile_pool(name="psum", bufs=7, space="PSUM"))
        opool = ctx.enter_context(tc.tile_pool(name="o", bufs=2))

        # ---- ACT table preload: dummy sign on scratch ----
        scr8 = const_pool.tile([P, 16], FP8, name="scr8")
        scrf = const_pool.tile([P, 16], F32, name="scrf")
        nc.vector.memset(scrf, 1.0)
        nc.scalar.sign(scr8, scrf)

        # ---- PE pre-warm: junk fp8 matmuls into a scratch psum bank ----
        wps = psumpool.tile([P, 512], F32, name="wps", tag="warm", bufs=1)
        warm = const_pool.tile([P, 512], FP8, name="warm")
        nc.vector.memset(warm, 1.0)
        for _ in range(N_WARM):
            nc.tensor.matmul(wps, warm[:, 0:P], warm,
                             start=True, stop=True, skip_group_check=True)

        wT2s = [const_pool.tile([P, KS * KS, CB, P], FP8, name=f"wT2_{ob}")
                for ob in range(OB)]
        smaps = [const_pool.tile([P, HW], BF16, name=f"smap{ob}")
                 for ob in range(OB)]

        # ---- banded padded sign images; pads zeroed once at startup ----
        # bufs[i][k] = band-k tile of double-buffer i
        im_bufs = []
        for i in range(2):
            tiles = []
            for k, (lo, hi) in enumerate(BANDS):
                n = hi - lo
                t_ = imgpool.tile([P, CB, _pad16(n * W1 + 1)], FP8,
                                  name=f"im{i}b{k}", tag=f"im{i}b{k}")
                tiles.append(t_)
                imr = t_[:, :, 0:n * W1].rearrange("p cb (r c) -> p cb r c", c=W1)
                r0 = 1 if lo == 0 else 0
                if lo == 0:
                    nc.gpsimd.memset(imr[:, :, 0, :], 0.0)        # top pad row
                nc.gpsimd.memset(imr[:, :, r0:n, 0], 0.0)         # left pads
                ktail = (n - 1) * W1 if hi == HP else n * W1
                nc.gpsimd.memset(t_[:, :, ktail:], 0.0)           # bottom/tail pads
            im_bufs.append(tiles)

        x_v = x_ap.rearrange("b (cb p) h w -> b p cb h w", p=P)
        out_v = out_ap  # [BL, OB, P, HW]

        def x_rows(k):
            lo, hi = BANDS[k]
            return max(lo - 1, 0), min(hi - 1, H)

        def issue_x_dma(b, xts, first=False):
            for k in range(len(BANDS)):
                xlo, xhi = x_rows(k)
                for cb in range(CB):
                    nc.sync.dma_start(xts[k][:, cb], x_v[b][:, cb, xlo:xhi, :])
                if first and k == 0:
                    nc.sync.dma_start(wT2s[0], w_ap[0])
                if first and k == 2:
                    nc.sync.dma_start(wT2s[1], w_ap[1])
            if first:
                nc.sync.dma_start(smaps[0], s_ap[0])
                nc.sync.dma_start(smaps[1], s_ap[1])

        def alloc_x():
            return [xpool.tile([P, CB, x_rows(k)[1] - x_rows(k)[0], H], F32,
                               name=f"xb{k}", tag=f"xb{k}")
                    for k in range(len(BANDS))]

        xts = [alloc_x()]
        issue_x_dma(0, xts[0], first=True)

        for b in range(BL):
            if b + 1 < BL:
                xts.append(alloc_x())
                issue_x_dma(b + 1, xts[b + 1])

            ims = im_bufs[b % 2]
            for k, (lo, hi) in enumerate(BANDS):
                n = hi - lo
                imr = ims[k][:, :, 0:n * W1].rearrange("p cb (r c) -> p cb r c", c=W1)
                r0 = 1 if lo == 0 else 0
                nrow = x_rows(k)[1] - x_rows(k)[0]
                for cb in range(CB):
                    nc.scalar.sign(imr[:, cb, r0:r0 + nrow, 1:1 + H],
                                   xts[b][k][:, cb])

            ostages = [opool.tile([P, HW], BF16, name=f"ostage{ob}",
                                  tag=f"ost{ob}") for ob in range(OB)]
            for t in range(T):
                k = BAND_OF_T[t]
                im = ims[k]
                rbase = t * R - BANDS[k][0]
                for ob in range(OB):
                    ps = psumpool.tile([P, R * W1], F32, name="cps", tag="cps")
                    for kk in range(KS * KS):
                        ky, kx = divmod(kk, KS)
                        off = (rbase + ky) * W1 + kx
                        nc.tensor.matmul(
                            ps[:, 0:NT],
                            wT2s[ob][:, kk],
                            im[:, :, off : off + NT],
                            start=(kk == 0),
                            stop=(kk == KS * KS - 1),
                            perf_mode=DR,
                        )
                    ps_v = ps.rearrange("p (r c) -> p r c", c=W1)[:, :, 0:H]
                    sl = smaps[ob][:, t * R * H : (t + 1) * R * H].rearrange(
                        "p (r c) -> p r c", c=H
                    )
                    ot = ostages[ob][:, t * R * H : (t + 1) * R * H].rearrange(
                        "p (r c) -> p r c", c=H
                    )
                    nc.vector.tensor_mul(ot, ps_v, sl)
                if t == T - 2:
                    for ob in range(OB):
                        nc.sync.dma_start(
                            out_v[b, ob, :, 0 : (T - 1) * R * H],
                            ostages[ob][:, 0 : (T - 1) * R * H],
                        )
                elif t == T - 1:
                    for ob in range(OB):
                        nc.sync.dma_start(
                            out_v[b, ob, :, (T - 1) * R * H : HW],
                            ostages[ob][:, (T - 1) * R * H : HW],
                        )


def build_nc(BL):
    nc = bacc.Bacc("TRN2", target_bir_lowering=False, debug=False)
    x = nc.dram_tensor("x", [BL, C, H, H], F32, kind="ExternalInput")
    w = nc.dram_tensor("wT2", [OB, P, KS * KS, CB, P], FP8, kind="ExternalInput")
    s = nc.dram_tensor("smap", [OB, P, HW], BF16, kind="ExternalInput")
    o = nc.dram_tensor("out", [BL, OB, P, HW], BF16, kind="ExternalOutput")
    with tile.TileContext(nc) as tc:
        build_conv(tc, o.ap(), x.ap(), w.ap(), s.ap(), BL)
    nc.compile()
    return nc


_nc_cache = {}


def _get_nc(BL):
    if BL not in _nc_cache:
        _nc_cache[BL] = build_nc(BL)
    return _nc_cache[BL]


def _build_inmaps(x, weight, alpha, beta, gamma):
    x = np.ascontiguousarray(np.asarray(x, dtype=np.float32))
    weight = np.asarray(weight, dtype=np.float32)
    alpha = np.asarray(alpha, dtype=np.float32)
    beta = np.asarray(beta, dtype=np.float32)
    gamma = np.asarray(gamma, dtype=np.float32)

    # sign(weight) packed transposed for DoubleRow: wT2[p, kk, ob, cb, o]
    s = np.where(weight > 0, np.float32(1.0), np.float32(-1.0))
    s_r = s.reshape(OB, P, CB, P, KS, KS)  # [ob, o, cb, p, ky, kx]
    wT2 = np.ascontiguousarray(s_r.transpose(0, 3, 4, 5, 2, 1).reshape(
        OB, P, KS * KS, CB, P)).astype(NP_FP8)

    # scale map alpha[o]*beta[y]*gamma[j] -> [OB, P, HW] (bf16: <0.4% rounding)
    smap = np.ascontiguousarray(
        (alpha * beta * gamma).astype(np.float32).reshape(OB, P, HW)).astype(
        mybir.dt.np(mybir.dt.bfloat16))

    BL = B // N_CORES
    xs = x.reshape(N_CORES, BL, C, H, H)
    return [
        {"x": xs[c], "wT2": wT2, "smap": smap}
        for c in range(N_CORES)
    ]


def kernel(x, weight, alpha, beta, gamma):
    BL = B // N_CORES
    nc = _get_nc(BL)
    in_maps = _build_inmaps(x, weight, alpha, beta, gamma)
    res = run_bass_kernel_spmd(nc, in_maps, list(range(N_CORES)))
    out = np.concatenate([r["out"] for r in res.results], axis=0)
    return np.ascontiguousarray(
        out.astype(np.float32).reshape(B, C, H, H))


# revision 17
# speedup vs baseline: 1.0070x; 1.0070x over previous
"""XNOR-Net++ 3x3 conv (sign(x) (*) sign(w) * alpha*beta*gamma) on 8 TRN2 NeuronCores.

Sharding: data-parallel over batch (32 -> 4 per core), weights/scales replicated.

Final design (trace-driven; baseline 177us -> 143 -> 128 -> 123 -> ~120):
- host packs sign(weight) into the transposed fp8 DoubleRow layout (split per
  ob so only the first half gates the start) and folds alpha*beta*gamma into
  per-ob bf16 scale maps (weight folding; <0.4% rounding vs 2e-2 tolerance,
  and exact 0 stays exact 0)
- zero-padded fp8 sign images with 57-stride rows (adjacent rows share a pad
  column): conv taps are flat offsets ky*57+kx, each matmul N=455 covers 8
  output rows; pads zeroed once at startup, signs write interiors only
- each image is split into seven row-band tiles (one per row-tile of 8
  output rows) with matching split x tiles; the tile framework tracks deps
  per tile, so the first matmul gates on 0.52MB of DMA + two 0.7us signs and
  afterwards bands arrive (1.3us DMA + 1.4us sign) faster than the 1.73us
  the PE spends per tile -- no pipeline gaps
- x for image b+1 is issued before image b's compute so the output triggers
  (which wait on epilogues) never delay it in the sync queue's FIFO
- the (tile, ob) loop is tile-outer, so each image band feeds 3.46us of
  matmuls and the sign chain stays well ahead of the PE
- outputs ship from bf16 staging tiles in 2 chunks per (image, ob) on the
  sync queue (the gpsimd queue's end-of-kernel DRAIN costs ~100ns/descriptor)
- PE pre-warm: junk fp8 matmuls bridge the head so the HAM clock gate is at
  2.4GHz when the real stream starts; a dummy sign preloads the ACT table
- PE then runs only the 504 conv matmuls back-to-back at ~192ns each
"""

from contextlib import ExitStack

import numpy as np

import concourse.bacc as bacc
import concourse.mybir as mybir
import concourse.tile as tile
from concourse.bass_utils import run_bass_kernel_spmd

N_CORES = 8
B, C, H, KS = 32, 256, 56, 3
P = 128
CB = C // P  # input-channel blocks (2)
OB = C // P  # output-channel blocks (2)
W1 = H + 1   # row stride (57): right pad of row r == left pad of row r+1
R = 8        # output rows per matmul tile
T = H // R   # row tiles per image (7)
NT = (R - 1) * W1 + H  # moving free dim per matmul (455)
HW = H * H   # pixels per image (3136)
HP = H + 2   # padded rows (58)

# image row-bands, one per row-tile: band t covers padded rows [8t, 8t+10)
BANDS = [(8 * t, 8 * t + 10) for t in range(T)]
BAND_OF_T = list(range(T))


def _pad16(n):
    return (n + 15) // 16 * 16


F32 = mybir.dt.float32
BF16 = mybir.dt.bfloat16
FP8 = mybir.dt.float8e4
DR = mybir.MatmulPerfMode.DoubleRow

NP_FP8 = mybir.dt.np(FP8)

N_WARM = 10  # pre-warm matmuls (N=512 fp8, ~4us at cold clock)


def build_conv(tc, out_ap, x_ap, w_ap, s_ap, BL):
    nc = tc.nc
    with ExitStack() as ctx:
        const_pool = ctx.enter_context(tc.tile_pool(name="const", bufs=1))
        xpool = ctx.enter_context(tc.tile_pool(name="x", bufs=2))
        imgpool = ctx.enter_context(tc.tile_pool(name="img", bufs=2))
        psumpool = ctx.enter_context(tc.tile_pool(name="psum", bufs=7, space="PSUM"))
        opool = ctx.enter_context(tc.tile_pool(name="o", bufs=2))

        # ---- ACT table preload: dummy sign on scratch ----
        scr8 = const_pool.tile([P, 16], FP8, name="scr8")
        scrf = const_pool.tile([P, 16], F32, name="scrf")
        nc.vector.memset(scrf, 1.0)
        nc.scalar.sign(scr8, scrf)

        # ---- PE pre-warm: junk fp8 matmuls into a scratch psum bank ----
        wps = psumpool.tile([P, 512], F32, name="wps", tag="warm", bufs=1)
        warm = const_pool.tile([P, 512], FP8, name="warm")
        nc.vector.memset(warm, 1.0)
        for _ in range(N_WARM):
            nc.tensor.matmul(wps, warm[:, 0:P], warm,
                             start=True, stop=True, skip_group_check=True)

        wT2s = [const_pool.tile([P, KS * KS, CB, P], FP8, name=f"wT2_{ob}")
                for ob in range(OB)]
        smaps = [const_pool.tile([P, HW], BF16, name=f"smap{ob}")
                 for ob in range(OB)]

        # ---- banded padded sign images; pads zeroed once at startup ----
        # bufs[i][k] = band-k tile of double-buffer i
        im_bufs = []
        for i in range(2):
            tiles = []
            for k, (lo, hi) in enumerate(BANDS):
                n = hi - lo
                t_ = imgpool.tile([P, CB, _pad16(n * W1 + 1)], FP8,
                                  name=f"im{i}b{k}", tag=f"im{i}b{k}")
                tiles.append(t_)
                imr = t_[:, :, 0:n * W1].rearrange("p cb (r c) -> p cb r c", c=W1)
                r0 = 1 if lo == 0 else 0
                if lo == 0:
                    nc.gpsimd.memset(imr[:, :, 0, :], 0.0)        # top pad row
                nc.gpsimd.memset(imr[:, :, r0:n, 0], 0.0)         # left pads
                ktail = (n - 1) * W1 if hi == HP else n * W1
                nc.gpsimd.memset(t_[:, :, ktail:], 0.0)           # bottom/tail pads
            im_bufs.append(tiles)

        x_v = x_ap.rearrange("b (cb p) h w -> b p cb h w", p=P)
        out_v = out_ap  # [BL, OB, P, HW]

        def x_rows(k):
            lo, hi = BANDS[k]
            return max(lo - 1, 0), min(hi - 1, H)

        def issue_x_dma(b, xts, first=False):
            for k in range(len(BANDS)):
                xlo, xhi = x_rows(k)
                for cb in range(CB):
                    nc.sync.dma_start(xts[k][:, cb], x_v[b][:, cb, xlo:xhi, :])
                if first and k == 0:
                    nc.sync.dma_start(wT2s[0], w_ap[0])
                if first and k == 2:
                    nc.sync.dma_start(wT2s[1], w_ap[1])
            if first:
                nc.sync.dma_start(smaps[0], s_ap[0])
                nc.sync.dma_start(smaps[1], s_ap[1])

        def alloc_x():
            return [xpool.tile([P, CB, x_rows(k)[1] - x_rows(k)[0], H], F32,
                               name=f"xb{k}", tag=f"xb{k}")
                    for k in range(len(BANDS))]

        xts = [alloc_x()]
        issue_x_dma(0, xts[0], first=True)

        for b in range(BL):
            if b + 1 < BL:
                xts.append(alloc_x())
                issue_x_dma(b + 1, xts[b + 1])

            ims = im_bufs[b % 2]
            for k, (lo, hi) in enumerate(BANDS):
                n = hi - lo
                imr = ims[k][:, :, 0:n * W1].rearrange("p cb (r c) -> p cb r c", c=W1)
                r0 = 1 if lo == 0 else 0
                nrow = x_rows(k)[1] - x_rows(k)[0]
                for cb in range(CB):
                    nc.scalar.sign(imr[:, cb, r0:r0 + nrow, 1:1 + H],
                                   xts[b][k][:, cb])

            ostages = [opool.tile([P, HW], BF16, name=f"ostage{ob}",
                                  tag=f"ost{ob}") for ob in range(OB)]
            for t in range(T):
                k = BAND_OF_T[t]
                im = ims[k]
                rbase = t * R - BANDS[k][0]
                for ob in range(OB):
                    ps = psumpool.tile([P, R * W1], F32, name="cps", tag="cps")
                    for kk in range(KS * KS):
                        ky, kx = divmod(kk, KS)
                        off = (rbase + ky) * W1 + kx
                        nc.tensor.matmul(
                            ps[:, 0:NT],
                            wT2s[ob][:, kk],
                            im[:, :, off : off + NT],
                            start=(kk == 0),
                            stop=(kk == KS * KS - 1),
                            perf_mode=DR,
                        )
                    ps_v = ps.rearrange("p (r c) -> p r c", c=W1)[:, :, 0:H]
                    sl = smaps[ob][:, t * R * H : (t + 1) * R * H].rearrange(
                        "p (r c) -> p r c", c=H
                    )
                    ot = ostages[ob][:, t * R * H : (t + 1) * R * H].rearrange(
                        "p (r c) -> p r c", c=H
                    )
                    nc.vector.tensor_mul(ot, ps_v, sl)
                if t == T - 2:
                    for ob in range(OB):
                        nc.sync.dma_start(
                            out_v[b, ob, :, 0 : (T - 1) * R * H],
                            ostages[ob][:, 0 : (T - 1) * R * H],
                        )
                elif t == T - 1:
                    for ob in range(OB):
                        nc.sync.dma_start(
                            out_v[b, ob, :, (T - 1) * R * H : HW],
                            ostages[ob][:, (T - 1) * R * H : HW],
                        )


def build_nc(BL):
    nc = bacc.Bacc("TRN2", target_bir_lowering=False, debug=False)
    x = nc.dram_tensor("x", [BL, C, H, H], F32, kind="ExternalInput")
    w = nc.dram_tensor("wT2", [OB, P, KS * KS, CB, P], FP8, kind="ExternalInput")
    s = nc.dram_tensor("smap", [OB, P, HW], BF16, kind="ExternalInput")
    o = nc.dram_tensor("out", [BL, OB, P, HW], BF16, kind="ExternalOutput")
    with tile.TileContext(nc) as tc:
        build_conv(tc, o.ap(), x.ap(), w.ap(), s.ap(), BL)
    nc.compile()
    return nc


_nc_cache = {}


def _get_nc(BL):
    if BL not in _nc_cache:
        _nc_cache[BL] = build_nc(BL)
    return _nc_cache[BL]


def _build_inmaps(x, weight, alpha, beta, gamma):
    x = np.ascontiguousarray(np.asarray(x, dtype=np.float32))
    weight = np.asarray(weight, dtype=np.float32)
    alpha = np.asarray(alpha, dtype=np.float32)
    beta = np.asarray(beta, dtype=np.float32)
    gamma = np.asarray(gamma, dtype=np.float32)

    # sign(weight) packed transposed for DoubleRow: wT2[p, kk, ob, cb, o]
    s = np.where(weight > 0, np.float32(1.0), np.float32(-1.0))
    s_r = s.reshape(OB, P, CB, P, KS, KS)  # [ob, o, cb, p, ky, kx]
    wT2 = np.ascontiguousarray(s_r.transpose(0, 3, 4, 5, 2, 1).reshape(
        OB, P, KS * KS, CB, P)).astype(NP_FP8)

    # scale map alpha[o]*beta[y]*gamma[j] -> [OB, P, HW] (bf16: <0.4% rounding)
    smap = np.ascontiguousarray(
        (alpha * beta * gamma).astype(np.float32).reshape(OB, P, HW)).astype(
        mybir.dt.np(mybir.dt.bfloat16))

    BL = B // N_CORES
    xs = x.reshape(N_CORES, BL, C, H, H)
    return [
        {"x": xs[c], "wT2": wT2, "smap": smap}
        for c in range(N_CORES)
    ]


def kernel(x, weight, alpha, beta, gamma):
    BL = B // N_CORES
    nc = _get_nc(BL)
    in_maps = _build_inmaps(x, weight, alpha, beta, gamma)
    res = run_bass_kernel_spmd(nc, in_maps, list(range(N_CORES)))
    out = np.concatenate([r["out"] for r in res.results], axis=0)
    return np.ascontiguousarray(
        out.astype(np.float32).reshape(B, C, H, H))


# revision 18
# speedup vs baseline: 1.0110x; 1.0040x over previous
"""XNOR-Net++ 3x3 conv (sign(x) (*) sign(w) * alpha*beta*gamma) on 8 TRN2 NeuronCores.

Sharding: data-parallel over batch (32 -> 4 per core), weights/scales replicated.

Final design (trace-driven; baseline 177us -> 143 -> 128 -> 123 -> ~120):
- host packs sign(weight) into the transposed fp8 DoubleRow layout (split per
  ob so only the first half gates the start) and folds alpha*beta*gamma into
  per-ob bf16 scale maps (weight folding; <0.4% rounding vs 2e-2 tolerance,
  and exact 0 stays exact 0)
- zero-padded fp8 sign images with 57-stride rows (adjacent rows share a pad
  column): conv taps are flat offsets ky*57+kx, each matmul N=455 covers 8
  output rows; pads zeroed once at startup, signs write interiors only
- each image is split into seven row-band tiles (one per row-tile of 8
  output rows) with matching split x tiles; the tile framework tracks deps
  per tile, so the first matmul gates on 0.52MB of DMA + two 0.7us signs and
  afterwards bands arrive (1.3us DMA + 1.4us sign) faster than the 1.73us
  the PE spends per tile -- no pipeline gaps
- x for image b+1 is issued before image b's compute so the output triggers
  (which wait on epilogues) never delay it in the sync queue's FIFO
- the (tile, ob) loop is tile-outer, so each image band feeds 3.46us of
  matmuls and the sign chain stays well ahead of the PE
- outputs ship from bf16 staging tiles in 2 chunks per (image, ob) on the
  sync queue (the gpsimd queue's end-of-kernel DRAIN costs ~100ns/descriptor)
- PE pre-warm: junk fp8 matmuls bridge the head so the HAM clock gate is at
  2.4GHz when the real stream starts; a dummy sign preloads the ACT table
- PE then runs only the 504 conv matmuls back-to-back at ~192ns each
"""

from contextlib import ExitStack

import numpy as np

import concourse.bacc as bacc
import concourse.mybir as mybir
import concourse.tile as tile
from concourse.bass_utils import run_bass_kernel_spmd

N_CORES = 8
B, C, H, KS = 32, 256, 56, 3
P = 128
CB = C // P  # input-channel blocks (2)
OB = C // P  # output-channel blocks (2)
W1 = H + 1   # row stride (57): right pad of row r == left pad of row r+1
R = 8        # output rows per matmul tile
T = H // R   # row tiles per image (7)
NT = (R - 1) * W1 + H  # moving free dim per matmul (455)
HW = H * H   # pixels per image (3136)
HP = H + 2   # padded rows (58)

# image row-bands, one per row-tile: band t covers padded rows [8t, 8t+10)
BANDS = [(8 * t, 8 * t + 10) for t in range(T)]
BAND_OF_T = list(range(T))


def _pad16(n):
    return (n + 15) // 16 * 16


F32 = mybir.dt.float32
BF16 = mybir.dt.bfloat16
FP8 = mybir.dt.float8e4
DR = mybir.MatmulPerfMode.DoubleRow

NP_FP8 = mybir.dt.np(FP8)

N_WARM = 10  # pre-warm matmuls (N=512 fp8, ~4us at cold clock)


def build_conv(tc, out_ap, x_ap, w_ap, s_ap, BL):
    nc = tc.nc
    with ExitStack() as ctx:
        const_pool = ctx.enter_context(tc.tile_pool(name="const", bufs=1))
        xpool = ctx.enter_context(tc.tile_pool(name="x", bufs=2))
        imgpool = ctx.enter_context(tc.tile_pool(name="img", bufs=2))
        psumpool = ctx.enter_context(tc.tile_pool(name="psum", bufs=7, space="PSUM"))
        opool = ctx.enter_context(tc.tile_pool(name="o", bufs=2))

        # ---- ACT table preload: dummy sign on scratch ----
        scr8 = const_pool.tile([P, 16], FP8, name="scr8")
        scrf = const_pool.tile([P, 16], F32, name="scrf")
        nc.vector.memset(scrf, 1.0)
        nc.scalar.sign(scr8, scrf)

        # ---- PE pre-warm: junk fp8 matmuls into a scratch psum bank ----
        wps = psumpool.tile([P, 512], F32, name="wps", tag="warm", bufs=1)
        warm = const_pool.tile([P, 512], FP8, name="warm")
        nc.vector.memset(warm, 1.0)
        for _ in range(N_WARM):
            nc.tensor.matmul(wps, warm[:, 0:P], warm,
                             start=True, stop=True, skip_group_check=True)

        wT2s = [const_pool.tile([P, KS * KS, CB, P], FP8, name=f"wT2_{ob}")
                for ob in range(OB)]
        smaps = [const_pool.tile([P, HW], BF16, name=f"smap{ob}")
                 for ob in range(OB)]

        # ---- banded padded sign images; pads zeroed once at startup ----
        # bufs[i][k] = band-k tile of double-buffer i
        im_bufs = []
        for i in range(2):
            tiles = []
            for k, (lo, hi) in enumerate(BANDS):
                n = hi - lo
                t_ = imgpool.tile([P, CB, _pad16(n * W1 + 1)], FP8,
                                  name=f"im{i}b{k}", tag=f"im{i}b{k}")
                tiles.append(t_)
                imr = t_[:, :, 0:n * W1].rearrange("p cb (r c) -> p cb r c", c=W1)
                r0 = 1 if lo == 0 else 0
                if lo == 0:
                    nc.gpsimd.memset(imr[:, :, 0, :], 0.0)        # top pad row
                nc.gpsimd.memset(imr[:, :, r0:n, 0], 0.0)         # left pads
                ktail = (n - 1) * W1 if hi == HP else n * W1
                nc.gpsimd.memset(t_[:, :, ktail:], 0.0)           # bottom/tail pads
            im_bufs.append(tiles)

        x_v = x_ap.rearrange("b (cb p) h w -> b p cb h w", p=P)
        out_v = out_ap  # [BL, OB, P, HW]

        def x_rows(k):
            lo, hi = BANDS[k]
            return max(lo - 1, 0), min(hi - 1, H)

        def issue_x_dma(b, xts, first=False):
            for k in range(len(BANDS)):
                xlo, xhi = x_rows(k)
                for cb in range(CB):
                    nc.sync.dma_start(xts[k][:, cb], x_v[b][:, cb, xlo:xhi, :])
                if first and k == 0:
                    nc.sync.dma_start(wT2s[0], w_ap[0])
                if first and k == 2:
                    nc.sync.dma_start(wT2s[1], w_ap[1])
            if first:
                nc.sync.dma_start(smaps[0], s_ap[0])
                nc.sync.dma_start(smaps[1], s_ap[1])

        def alloc_x():
            return [xpool.tile([P, CB, x_rows(k)[1] - x_rows(k)[0], H], F32,
                               name=f"xb{k}", tag=f"xb{k}")
                    for k in range(len(BANDS))]

        xts = [alloc_x()]
        issue_x_dma(0, xts[0], first=True)

        for b in range(BL):
            if b + 1 < BL:
                xts.append(alloc_x())
                issue_x_dma(b + 1, xts[b + 1])

            ims = im_bufs[b % 2]
            for k, (lo, hi) in enumerate(BANDS):
                n = hi - lo
                imr = ims[k][:, :, 0:n * W1].rearrange("p cb (r c) -> p cb r c", c=W1)
                r0 = 1 if lo == 0 else 0
                nrow = x_rows(k)[1] - x_rows(k)[0]
                for cb in range(CB):
                    nc.scalar.sign(imr[:, cb, r0:r0 + nrow, 1:1 + H],
                                   xts[b][k][:, cb])

            ostages = [opool.tile([P, HW], BF16, name=f"ostage{ob}",
                                  tag=f"ost{ob}") for ob in range(OB)]
            for t in range(T):
                k = BAND_OF_T[t]
                im = ims[k]
                rbase = t * R - BANDS[k][0]
                for ob in range(OB):
                    ps = psumpool.tile([P, R * W1], F32, name="cps", tag="cps")
                    for kk in range(KS * KS):
                        ky, kx = divmod(kk, KS)
                        off = (rbase + ky) * W1 + kx
                        nc.tensor.matmul(
                            ps[:, 0:NT],
                            wT2s[ob][:, kk],
                            im[:, :, off : off + NT],
                            start=(kk == 0),
                            stop=(kk == KS * KS - 1),
                            perf_mode=DR,
                        )
                    ps_v = ps.rearrange("p (r c) -> p r c", c=W1)[:, :, 0:H]
                    sl = smaps[ob][:, t * R * H : (t + 1) * R * H].rearrange(
                        "p (r c) -> p r c", c=H
                    )
                    ot = ostages[ob][:, t * R * H : (t + 1) * R * H].rearrange(
                        "p (r c) -> p r c", c=H
                    )
                    nc.vector.tensor_mul(ot, ps_v, sl)
                if t == T - 2:
                    for ob in range(OB):
                        nc.sync.dma_start(
                            out_v[b, ob, :, 0 : (T - 1) * R * H],
                            ostages[ob][:, 0 : (T - 1) * R * H],
                        )
                elif t == T - 1:
                    for ob in range(OB):
                        nc.sync.dma_start(
                            out_v[b, ob, :, (T - 1) * R * H : HW],
                            ostages[ob][:, (T - 1) * R * H : HW],
                        )


def build_nc(BL):
    nc = bacc.Bacc("TRN2", target_bir_lowering=False, debug=False)
    x = nc.dram_tensor("x", [BL, C, H, H], F32, kind="ExternalInput")
    w = nc.dram_tensor("wT2", [OB, P, KS * KS, CB, P], FP8, kind="ExternalInput")
    s = nc.dram_tensor("smap", [OB, P, HW], BF16, kind="ExternalInput")
    o = nc.dram_tensor("out", [BL, OB, P, HW], BF16, kind="ExternalOutput")
    with tile.TileContext(nc) as tc:
        build_conv(tc, o.ap(), x.ap(), w.ap(), s.ap(), BL)
    nc.compile()
    return nc


_nc_cache = {}


def _get_nc(BL):
    if BL not in _nc_cache:
        _nc_cache[BL] = build_nc(BL)
    return _nc_cache[BL]


def _build_inmaps(x, weight, alpha, beta, gamma):
    x = np.ascontiguousarray(np.asarray(x, dtype=np.float32))
    weight = np.asarray(weight, dtype=np.float32)
    alpha = np.asarray(alpha, dtype=np.float32)
    beta = np.asarray(beta, dtype=np.float32)
    gamma = np.asarray(gamma, dtype=np.float32)

    # sign(weight) packed transposed for DoubleRow: wT2[ob, p, kk, cb, o]
    s = np.where(weight > 0, np.float32(1.0), np.float32(-1.0))
    s_r = s.reshape(OB, P, CB, P, KS, KS)  # [ob, o, cb, p, ky, kx]
    wT2 = np.ascontiguousarray(s_r.transpose(0, 3, 4, 5, 2, 1).reshape(
        OB, P, KS * KS, CB, P)).astype(NP_FP8)

    # scale map alpha[o]*beta[y]*gamma[j] -> [OB, P, HW] (bf16: <0.4% rounding)
    smap = np.ascontiguousarray(
        (alpha * beta * gamma).astype(np.float32).reshape(OB, P, HW)).astype(
        mybir.dt.np(mybir.dt.bfloat16))

    BL = B // N_CORES
    xs = x.reshape(N_CORES, BL, C, H, H)
    return [
        {"x": xs[c], "wT2": wT2, "smap": smap}
        for c in range(N_CORES)
    ]


def kernel(x, weight, alpha, beta, gamma):
    BL = B // N_CORES
    nc = _get_nc(BL)
    in_maps = _build_inmaps(x, weight, alpha, beta, gamma)
    res = run_bass_kernel_spmd(nc, in_maps, list(range(N_CORES)))
    out = np.concatenate([r["out"] for r in res.results], axis=0)
    return np.ascontiguousarray(
        out.astype(np.float32).reshape(B, C, H, H))


# revision 19
# speedup vs baseline: 1.0184x; 1.0073x over previous
"""XNOR-Net++ 3x3 conv (sign(x) (*) sign(w) * alpha*beta*gamma) on 8 TRN2 NeuronCores.

Sharding: data-parallel over batch (32 -> 4 per core), weights/scales replicated.

Final design (trace-driven; baseline 177us -> 143 -> 128 -> 123 -> ~120):
- host packs sign(weight) into the transposed fp8 DoubleRow layout (split per
  ob so only the first half gates the start) and folds alpha*beta*gamma into
  per-ob bf16 scale maps (weight folding; <0.4% rounding vs 2e-2 tolerance,
  and exact 0 stays exact 0)
- zero-padded fp8 sign images with 57-stride rows (adjacent rows share a pad
  column): conv taps are flat offsets ky*57+kx, each matmul N=455 covers 8
  output rows; pads zeroed once at startup, signs write interiors only
- each image is split into seven row-band tiles (one per row-tile of 8
  output rows) with matching split x tiles; the tile framework tracks deps
  per tile, so the first matmul gates on 0.52MB of DMA + two 0.7us signs and
  afterwards bands arrive (1.3us DMA + 1.4us sign) faster than the 1.73us
  the PE spends per tile -- no pipeline gaps
- x for image b+1 is issued before image b's compute so the output triggers
  (which wait on epilogues) never delay it in the sync queue's FIFO
- the (tile, ob) loop is tile-outer, so each image band feeds 3.46us of
  matmuls and the sign chain stays well ahead of the PE
- outputs ship from bf16 staging tiles in 2 chunks per (image, ob) on the
  sync queue (the gpsimd queue's end-of-kernel DRAIN costs ~100ns/descriptor)
- PE pre-warm: junk fp8 matmuls bridge the head so the HAM clock gate is at
  2.4GHz when the real stream starts; a dummy sign preloads the ACT table
- PE then runs only the 504 conv matmuls back-to-back at ~192ns each
"""

from contextlib import ExitStack

import numpy as np

import concourse.bacc as bacc
import concourse.mybir as mybir
import concourse.tile as tile
from concourse.bass_utils import run_bass_kernel_spmd

N_CORES = 8
B, C, H, KS = 32, 256, 56, 3
P = 128
CB = C // P  # input-channel blocks (2)
OB = C // P  # output-channel blocks (2)
W1 = H + 1   # row stride (57): right pad of row r == left pad of row r+1
R = 8        # output rows per matmul tile
T = H // R   # row tiles per image (7)
NT = (R - 1) * W1 + H  # moving free dim per matmul (455)
HW = H * H   # pixels per image (3136)
HP = H + 2   # padded rows (58)

# image row-bands, one per row-tile: band t covers padded rows [8t, 8t+10)
BANDS = [(8 * t, 8 * t + 10) for t in range(T)]
BAND_OF_T = list(range(T))


def _pad16(n):
    return (n + 15) // 16 * 16


F32 = mybir.dt.float32
BF16 = mybir.dt.bfloat16
FP8 = mybir.dt.float8e4
DR = mybir.MatmulPerfMode.DoubleRow

NP_FP8 = mybir.dt.np(FP8)

N_WARM = 10  # pre-warm matmuls (N=512 fp8, ~4us at cold clock)


def build_conv(tc, out_ap, x_ap, w_ap, s_ap, BL):
    nc = tc.nc
    with ExitStack() as ctx:
        const_pool = ctx.enter_context(tc.tile_pool(name="const", bufs=1))
        xpool = ctx.enter_context(tc.tile_pool(name="x", bufs=2))
        imgpool = ctx.enter_context(tc.tile_pool(name="img", bufs=2))
        psumpool = ctx.enter_context(tc.tile_pool(name="psum", bufs=7, space="PSUM"))
        opool = ctx.enter_context(tc.tile_pool(name="o", bufs=2))

        # ---- ACT table preload: dummy sign on scratch ----
        scr8 = const_pool.tile([P, 16], FP8, name="scr8")
        scrf = const_pool.tile([P, 16], F32, name="scrf")
        nc.vector.memset(scrf, 1.0)
        nc.scalar.sign(scr8, scrf)

        # ---- PE pre-warm: junk fp8 matmuls into a scratch psum bank ----
        wps = psumpool.tile([P, 512], F32, name="wps", tag="warm", bufs=1)
        warm = const_pool.tile([P, 512], FP8, name="warm")
        nc.vector.memset(warm, 1.0)
        for _ in range(N_WARM):
            nc.tensor.matmul(wps, warm[:, 0:P], warm,
                             start=True, stop=True, skip_group_check=True)

        wT2s = [const_pool.tile([P, KS * KS, CB, P], FP8, name=f"wT2_{ob}")
                for ob in range(OB)]
        smaps = [const_pool.tile([P, HW], BF16, name=f"smap{ob}")
                 for ob in range(OB)]

        # ---- banded padded sign images; pads zeroed once at startup ----
        # bufs[i][k] = band-k tile of double-buffer i
        im_bufs = []
        for i in range(2):
            tiles = []
            for k, (lo, hi) in enumerate(BANDS):
                n = hi - lo
                t_ = imgpool.tile([P, CB, _pad16(n * W1 + 1)], FP8,
                                  name=f"im{i}b{k}", tag=f"im{i}b{k}")
                tiles.append(t_)
                imr = t_[:, :, 0:n * W1].rearrange("p cb (r c) -> p cb r c", c=W1)
                r0 = 1 if lo == 0 else 0
                if lo == 0:
                    nc.gpsimd.memset(imr[:, :, 0, :], 0.0)        # top pad row
                nc.gpsimd.memset(imr[:, :, r0:n, 0], 0.0)         # left pads
                ktail = (n - 1) * W1 if hi == HP else n * W1
                nc.gpsimd.memset(t_[:, :, ktail:], 0.0)           # bottom/tail pads
            im_bufs.append(tiles)

        x_v = x_ap.rearrange("b (cb p) h w -> b p cb h w", p=P)
        out_v = out_ap  # [BL, OB, P, HW]

        def x_rows(k):
            lo, hi = BANDS[k]
            return max(lo - 1, 0), min(hi - 1, H)

        def issue_x_dma(b, xts, first=False):
            for k in range(len(BANDS)):
                xlo, xhi = x_rows(k)
                for cb in range(CB):
                    nc.sync.dma_start(xts[k][:, cb], x_v[b][:, cb, xlo:xhi, :])
                if first and k == 0:
                    nc.sync.dma_start(wT2s[0], w_ap[0])
                if first and k == 2:
                    nc.sync.dma_start(wT2s[1], w_ap[1])
            if first:
                nc.sync.dma_start(smaps[0], s_ap[0])
                nc.sync.dma_start(smaps[1], s_ap[1])

        def alloc_x():
            return [xpool.tile([P, CB, x_rows(k)[1] - x_rows(k)[0], H], BF16,
                               name=f"xb{k}", tag=f"xb{k}")
                    for k in range(len(BANDS))]

        xts = [alloc_x()]
        issue_x_dma(0, xts[0], first=True)

        for b in range(BL):
            if b + 1 < BL:
                xts.append(alloc_x())
                issue_x_dma(b + 1, xts[b + 1])

            ims = im_bufs[b % 2]
            for k, (lo, hi) in enumerate(BANDS):
                n = hi - lo
                imr = ims[k][:, :, 0:n * W1].rearrange("p cb (r c) -> p cb r c", c=W1)
                r0 = 1 if lo == 0 else 0
                nrow = x_rows(k)[1] - x_rows(k)[0]
                for cb in range(CB):
                    nc.scalar.sign(imr[:, cb, r0:r0 + nrow, 1:1 + H],
                                   xts[b][k][:, cb])

            ostages = [opool.tile([P, HW], BF16, name=f"ostage{ob}",
                                  tag=f"ost{ob}") for ob in range(OB)]
            for t in range(T):
                k = BAND_OF_T[t]
                im = ims[k]
                rbase = t * R - BANDS[k][0]
                for ob in range(OB):
                    ps = psumpool.tile([P, R * W1], F32, name="cps", tag="cps")
                    for kk in range(KS * KS):
                        ky, kx = divmod(kk, KS)
                        off = (rbase + ky) * W1 + kx
                        nc.tensor.matmul(
                            ps[:, 0:NT],
                            wT2s[ob][:, kk],
                            im[:, :, off : off + NT],
                            start=(kk == 0),
                            stop=(kk == KS * KS - 1),
                            perf_mode=DR,
                        )
                    ps_v = ps.rearrange("p (r c) -> p r c", c=W1)[:, :, 0:H]
                    sl = smaps[ob][:, t * R * H : (t + 1) * R * H].rearrange(
                        "p (r c) -> p r c", c=H
                    )
                    ot = ostages[ob][:, t * R * H : (t + 1) * R * H].rearrange(
                        "p (r c) -> p r c", c=H
                    )
                    nc.vector.tensor_mul(ot, ps_v, sl)
                if t == T - 2:
                    for ob in range(OB):
                        nc.sync.dma_start(
                            out_v[b, ob, :, 0 : (T - 1) * R * H],
                            ostages[ob][:, 0 : (T - 1) * R * H],
                        )
                elif t == T - 1:
                    for ob in range(OB):
                        nc.sync.dma_start(
                            out_v[b, ob, :, (T - 1) * R * H : HW],
                            ostages[ob][:, (T - 1) * R * H : HW],
                        )


def build_nc(BL):
    nc = bacc.Bacc("TRN2", target_bir_lowering=False, debug=False)
    x = nc.dram_tensor("x", [BL, C, H, H], BF16, kind="ExternalInput")
    w = nc.dram_tensor("wT2", [OB, P, KS * KS, CB, P], FP8, kind="ExternalInput")
    s = nc.dram_tensor("smap", [OB, P, HW], BF16, kind="ExternalInput")
    o = nc.dram_tensor("out", [BL, OB, P, HW], BF16, kind="ExternalOutput")
    with tile.TileContext(nc) as tc:
        build_conv(tc, o.ap(), x.ap(), w.ap(), s.ap(), BL)
    nc.compile()
    return nc


_nc_cache = {}


def _get_nc(BL):
    if BL not in _nc_cache:
        _nc_cache[BL] = build_nc(BL)
    return _nc_cache[BL]


def _build_inmaps(x, weight, alpha, beta, gamma):
    # bf16 halves the x DMA; the cast preserves every sign bit, and sign(x)
    # is the only use of x (device still binarizes)
    x = np.ascontiguousarray(
        np.asarray(x, dtype=np.float32).astype(mybir.dt.np(mybir.dt.bfloat16)))
    weight = np.asarray(weight, dtype=np.float32)
    alpha = np.asarray(alpha, dtype=np.float32)
    beta = np.asarray(beta, dtype=np.float32)
    gamma = np.asarray(gamma, dtype=np.float32)

    # sign(weight) packed transposed for DoubleRow: wT2[ob, p, kk, cb, o]
    s = np.where(weight > 0, np.float32(1.0), np.float32(-1.0))
    s_r = s.reshape(OB, P, CB, P, KS, KS)  # [ob, o, cb, p, ky, kx]
    wT2 = np.ascontiguousarray(s_r.transpose(0, 3, 4, 5, 2, 1).reshape(
        OB, P, KS * KS, CB, P)).astype(NP_FP8)

    # scale map alpha[o]*beta[y]*gamma[j] -> [OB, P, HW] (bf16: <0.4% rounding)
    smap = np.ascontiguousarray(
        (alpha * beta * gamma).astype(np.float32).reshape(OB, P, HW)).astype(
        mybir.dt.np(mybir.dt.bfloat16))

    BL = B // N_CORES
    xs = x.reshape(N_CORES, BL, C, H, H)
    return [
        {"x": xs[c], "wT2": wT2, "smap": smap}
        for c in range(N_CORES)
    ]


def kernel(x, weight, alpha, beta, gamma):
    BL = B // N_CORES
    nc = _get_nc(BL)
    in_maps = _build_inmaps(x, weight, alpha, beta, gamma)
    res = run_bass_kernel_spmd(nc, in_maps, list(range(N_CORES)))
    out = np.concatenate([r["out"] for r in res.results], axis=0)
    return np.ascontiguousarray(
        out.astype(np.float32).reshape(B, C, H, H))


# revision 21
# speedup vs baseline: 1.0495x; 1.0306x over previous
"""XNOR-Net++ 3x3 conv (sign(x) (*) sign(w) * alpha*beta*gamma) on 8 TRN2 NeuronCores.

Sharding: data-parallel over batch (32 -> 4 per core), weights/scales replicated.

Final design (trace-driven; baseline 177us -> 143 -> 128 -> 123 -> ~117):
- host packs sign(weight) into the transposed fp8 DoubleRow layout (split per
  ob so only the first half gates the start) and folds alpha*beta*gamma into
  per-ob bf16 scale maps (weight folding; <0.4% rounding vs 2e-2 tolerance,
  and exact 0 stays exact 0)
- zero-padded fp8 sign images with 57-stride rows (adjacent rows share a pad
  column): conv taps are flat offsets ky*57+kx, each matmul N=455 covers 8
  output rows; pads zeroed once at startup, signs write interiors only
- x ships to the device as bf16 (sign bits preserved exactly; halves the
  12.8MB/core input DMA; the device still binarizes)
- each image is split into seven row-band tiles (one per row-tile of 8
  output rows) with matching split x tiles; the tile framework tracks deps
  per tile, so the first matmul gates on 0.52MB of DMA + two 0.7us signs and
  afterwards bands arrive (1.3us DMA + 1.4us sign) faster than the 1.73us
  the PE spends per tile -- no pipeline gaps
- x for image b+1 is issued before image b's compute so the output triggers
  (which wait on epilogues) never delay it in the sync queue's FIFO
- the (tile, ob) loop is tile-outer, so each image band feeds 3.46us of
  matmuls and the sign chain stays well ahead of the PE
- outputs ship from bf16 staging tiles in 2 chunks per (image, ob) on the
  sync queue (the gpsimd queue's end-of-kernel DRAIN costs ~100ns/descriptor)
- PE pre-warm: junk fp8 matmuls bridge the head so the HAM clock gate is at
  2.4GHz when the real stream starts; a dummy sign preloads the ACT table
- PE then runs only the 504 conv matmuls back-to-back at ~192ns each
"""

from contextlib import ExitStack

import numpy as np

import concourse.bacc as bacc
import concourse.mybir as mybir
import concourse.tile as tile
from concourse.bass_utils import run_bass_kernel_spmd

N_CORES = 8
B, C, H, KS = 32, 256, 56, 3
P = 128
CB = C // P  # input-channel blocks (2)
OB = C // P  # output-channel blocks (2)
W1 = H + 1   # row stride (57): right pad of row r == left pad of row r+1
R = 8        # output rows per matmul tile
T = H // R   # row tiles per image (7)
NT = (R - 1) * W1 + H  # moving free dim per matmul (455)
HW = H * H   # pixels per image (3136)
HP = H + 2   # padded rows (58)

# image row-bands, one per row-tile: band t covers padded rows [8t, 8t+10)
BANDS = [(8 * t, 8 * t + 10) for t in range(T)]
BAND_OF_T = list(range(T))


def _pad16(n):
    return (n + 15) // 16 * 16


F32 = mybir.dt.float32
BF16 = mybir.dt.bfloat16
FP8 = mybir.dt.float8e4
DR = mybir.MatmulPerfMode.DoubleRow

NP_FP8 = mybir.dt.np(FP8)

N_WARM = 10  # pre-warm matmuls (N=512 fp8, ~4us at cold clock)


def build_conv(tc, out_ap, x_ap, w_ap, s_ap, BL):
    nc = tc.nc
    with ExitStack() as ctx:
        const_pool = ctx.enter_context(tc.tile_pool(name="const", bufs=1))
        xpool = ctx.enter_context(tc.tile_pool(name="x", bufs=2))
        imgpool = ctx.enter_context(tc.tile_pool(name="img", bufs=2))
        psumpool = ctx.enter_context(tc.tile_pool(name="psum", bufs=7, space="PSUM"))
        opool = ctx.enter_context(tc.tile_pool(name="o", bufs=2))

        # ---- ACT table preload: dummy sign on scratch ----
        scr8 = const_pool.tile([P, 16], FP8, name="scr8")
        scrf = const_pool.tile([P, 16], F32, name="scrf")
        nc.vector.memset(scrf, 1.0)
        nc.scalar.sign(scr8, scrf)

        # ---- PE pre-warm: junk fp8 matmuls into a scratch psum bank ----
        wps = psumpool.tile([P, 512], F32, name="wps", tag="warm", bufs=1)
        warm = const_pool.tile([P, 512], FP8, name="warm")
        nc.vector.memset(warm, 1.0)
        for _ in range(N_WARM):
            nc.tensor.matmul(wps, warm[:, 0:P], warm,
                             start=True, stop=True, skip_group_check=True)

        wT2s = [const_pool.tile([P, KS * KS, CB, P], FP8, name=f"wT2_{ob}")
                for ob in range(OB)]
        smaps = [const_pool.tile([P, HW], BF16, name=f"smap{ob}")
                 for ob in range(OB)]

        # ---- banded padded sign images; pads zeroed once at startup ----
        # bufs[i][k] = band-k tile of double-buffer i
        im_bufs = []
        for i in range(2):
            tiles = []
            for k, (lo, hi) in enumerate(BANDS):
                n = hi - lo
                t_ = imgpool.tile([P, CB, _pad16(n * W1 + 1)], FP8,
                                  name=f"im{i}b{k}", tag=f"im{i}b{k}")
                tiles.append(t_)
                imr = t_[:, :, 0:n * W1].rearrange("p cb (r c) -> p cb r c", c=W1)
                r0 = 1 if lo == 0 else 0
                if lo == 0:
                    nc.gpsimd.memset(imr[:, :, 0, :], 0.0)        # top pad row
                nc.gpsimd.memset(imr[:, :, r0:n, 0], 0.0)         # left pads
                ktail = (n - 1) * W1 if hi == HP else n * W1
                nc.gpsimd.memset(t_[:, :, ktail:], 0.0)           # bottom/tail pads
            im_bufs.append(tiles)

        x_v = x_ap.rearrange("b (cb p) h w -> b p cb h w", p=P)
        out_v = out_ap  # [BL, OB, P, HW]

        def x_rows(k):
            lo, hi = BANDS[k]
            return max(lo - 1, 0), min(hi - 1, H)

        def issue_x_dma(b, xts, first=False):
            for k in range(len(BANDS)):
                xlo, xhi = x_rows(k)
                for cb in range(CB):
                    nc.sync.dma_start(xts[k][:, cb], x_v[b][:, cb, xlo:xhi, :])
                if first and k == 0:
                    # both weight halves up front: with the tile-outer loop,
                    # ob1 is needed ~1.7us after ob0
                    nc.sync.dma_start(wT2s[0], w_ap[0])
                    nc.sync.dma_start(wT2s[1], w_ap[1])
            if first:
                nc.sync.dma_start(smaps[0], s_ap[0])
                nc.sync.dma_start(smaps[1], s_ap[1])

        def alloc_x():
            return [xpool.tile([P, CB, x_rows(k)[1] - x_rows(k)[0], H], BF16,
                               name=f"xb{k}", tag=f"xb{k}")
                    for k in range(len(BANDS))]

        xts = [alloc_x()]
        issue_x_dma(0, xts[0], first=True)

        for b in range(BL):
            if b + 1 < BL:
                xts.append(alloc_x())
                issue_x_dma(b + 1, xts[b + 1])

            ims = im_bufs[b % 2]
            for k, (lo, hi) in enumerate(BANDS):
                n = hi - lo
                imr = ims[k][:, :, 0:n * W1].rearrange("p cb (r c) -> p cb r c", c=W1)
                r0 = 1 if lo == 0 else 0
                nrow = x_rows(k)[1] - x_rows(k)[0]
                for cb in range(CB):
                    nc.scalar.sign(imr[:, cb, r0:r0 + nrow, 1:1 + H],
                                   xts[b][k][:, cb])

            ostages = [opool.tile([P, HW], BF16, name=f"ostage{ob}",
                                  tag=f"ost{ob}") for ob in range(OB)]
            for t in range(T):
                k = BAND_OF_T[t]
                im = ims[k]
                rbase = t * R - BANDS[k][0]
                for ob in range(OB):
                    ps = psumpool.tile([P, R * W1], F32, name="cps", tag="cps")
                    for kk in range(KS * KS):
                        ky, kx = divmod(kk, KS)
                        off = (rbase + ky) * W1 + kx
                        nc.tensor.matmul(
                            ps[:, 0:NT],
                            wT2s[ob][:, kk],
                            im[:, :, off : off + NT],
                            start=(kk == 0),
                            stop=(kk == KS * KS - 1),
                            perf_mode=DR,
                        )
                    ps_v = ps.rearrange("p (r c) -> p r c", c=W1)[:, :, 0:H]
                    sl = smaps[ob][:, t * R * H : (t + 1) * R * H].rearrange(
                        "p (r c) -> p r c", c=H
                    )
                    ot = ostages[ob][:, t * R * H : (t + 1) * R * H].rearrange(
                        "p (r c) -> p r c", c=H
                    )
                    nc.vector.tensor_mul(ot, ps_v, sl)
                if t == T - 2:
                    for ob in range(OB):
                        nc.sync.dma_start(
                            out_v[b, ob, :, 0 : (T - 1) * R * H],
                            ostages[ob][:, 0 : (T - 1) * R * H],
                        )
                elif t == T - 1:
                    for ob in range(OB):
                        nc.sync.dma_start(
                            out_v[b, ob, :, (T - 1) * R * H : HW],
                            ostages[ob][:, (T - 1) * R * H : HW],
                        )


def build_nc(BL):
    nc = bacc.Bacc("TRN2", target_bir_lowering=False, debug=False)
    x = nc.dram_tensor("x", [BL, C, H, H], BF16, kind="ExternalInput")
    w = nc.dram_tensor("wT2", [OB, P, KS * KS, CB, P], FP8, kind="ExternalInput")
    s = nc.dram_tensor("smap", [OB, P, HW], BF16, kind="ExternalInput")
    o = nc.dram_tensor("out", [BL, OB, P, HW], BF16, kind="ExternalOutput")
    with tile.TileContext(nc) as tc:
        build_conv(tc, o.ap(), x.ap(), w.ap(), s.ap(), BL)
    nc.compile()
    return nc


_nc_cache = {}


def _get_nc(BL):
    if BL not in _nc_cache:
        _nc_cache[BL] = build_nc(BL)
    return _nc_cache[BL]


def _build_inmaps(x, weight, alpha, beta, gamma):
    # bf16 halves the x DMA; the cast preserves every sign bit, and sign(x)
    # is the only use of x (device still binarizes)
    x = np.ascontiguousarray(
        np.asarray(x, dtype=np.float32).astype(mybir.dt.np(mybir.dt.bfloat16)))
    weight = np.asarray(weight, dtype=np.float32)
    alpha = np.asarray(alpha, dtype=np.float32)
    beta = np.asarray(beta, dtype=np.float32)
    gamma = np.asarray(gamma, dtype=np.float32)

    # sign(weight) packed transposed for DoubleRow: wT2[ob, p, kk, cb, o]
    s = np.where(weight > 0, np.float32(1.0), np.float32(-1.0))
    s_r = s.reshape(OB, P, CB, P, KS, KS)  # [ob, o, cb, p, ky, kx]
    wT2 = np.ascontiguousarray(s_r.transpose(0, 3, 4, 5, 2, 1).reshape(
        OB, P, KS * KS, CB, P)).astype(NP_FP8)

    # scale map alpha[o]*beta[y]*gamma[j] -> [OB, P, HW] (bf16: <0.4% rounding)
    smap = np.ascontiguousarray(
        (alpha * beta * gamma).astype(np.float32).reshape(OB, P, HW)).astype(
        mybir.dt.np(mybir.dt.bfloat16))

    BL = B // N_CORES
    xs = x.reshape(N_CORES, BL, C, H, H)
    return [
        {"x": xs[c], "wT2": wT2, "smap": smap}
        for c in range(N_CORES)
    ]


def kernel(x, weight, alpha, beta, gamma):
    BL = B // N_CORES
    nc = _get_nc(BL)
    in_maps = _build_inmaps(x, weight, alpha, beta, gamma)
    res = run_bass_kernel_spmd(nc, in_maps, list(range(N_CORES)))
    out = np.concatenate([r["out"] for r in res.results], axis=0)
    return np.ascontiguousarray(
        out.astype(np.float32).reshape(B, C, H, H))
